# revision 4
# baseline (speedup 1.0000x reference)
"""BalancedWeightClusterLoss on 8 Trainium2 NeuronCores (Bass/Tile).

Reference computation (per channel c of weight [C, K], scale [C]):
    mean, std(ddof=1) over K
    lower = mean - 2*std ; step = 4*std/15
    idx = clip((w - lower)/step, 0, 14) -> int (trunc == floor here)
    target = scale * (idx - 7)
    loss = sum |w - target|

Kernel derivation (per channel; r = 1/step, nb1 = mean*r - 7):
    idx = floor((w-lower)*r) = round(w*r - nb1)      (round(x-.5)==floor(x))
    jc7 = clip(round(z), 0, 14) - 7,  z = w*r - nb1
    loss = sum |w - s*jc7|

Engine split (both ~26us/block, pipelined across 4 row-blocks):
    ACT:  Copy(w_f32 -> w_f16) + riding accumulator => Sum(w)
          Square(w_f32, cols [0:SQA]) + accumulator => most of Sum(w^2)
          Sqrt for step (tiny)
    DVE:  SQSUM custom op on cols [SQA:K] of w_f16  => rest of Sum(w^2)
          z = w_f16*r - nb1     (tensor_scalar dual, f16, 4x mode)
          VQ_LOSS_ANT custom op (registered at import):
             j = min(max(z + 2^23, 2^23) - (2^23+7), 7)  [f32 internal round]
             out = |w - s*j|, accum_out = per-channel loss sum
          (1 elem/cycle; the loss reduction rides the same pass for free)
    All big reductions ride engine accumulators; no separate reduce passes.
    Square-pass scratch and custom-op garbage both land in the z tile.

Sharding: channels 4096 -> 512 per core (8 cores) x 4 row-blocks of 128
partitions. w is read from HBM exactly once (memory roofline ~94us/core).
Host sums the 8 x [128, 4] partial losses in float64.
"""
import numpy as np

import concourse.bacc as bacc
import concourse.tile as tile
from concourse import mybir
from concourse.bass_utils import run_bass_kernel_spmd

f32 = mybir.dt.float32
f16 = mybir.dt.float16
Alu = mybir.AluOpType
Act = mybir.ActivationFunctionType

# problem shape (hardcoded per contest contract)
CFULL, K = 4096, 16384
NCORES = 8
CSH = CFULL // NCORES          # 512 channels per core
P = 128                        # SBUF partitions
NBLK = CSH // P                # 4 row-blocks per core
CH = 8192                      # phase-1 chunk (f32 DMA + ACT passes)
NCH = K // CH                  # 2
SQA = 13312                    # Sum(w^2): ACT takes [0:SQA], DVE [SQA:K]

RND = float(2 ** 23)           # f32 round-to-int bias
RND7 = float(2 ** 23 + 7)
INV_K = 1.0 / K
# step^2 = K2 * var_biased ; var_b = E[w^2] - mean^2
K2 = (4.0 / 15.0) ** 2 * (K / (K - 1.0))

_PROGRAM = None


def _vq_ref(in0, in1, c0, c1, c2):
    """numpy reference for VQ_LOSS_ANT (CoreSim executes this)."""
    z32 = np.asarray(in0, np.float32)
    v = (z32 + np.float32(c0)).astype(np.float32)
    v2 = np.maximum(v, np.float32(c0))
    j0 = (v2 - np.float32(c2)).astype(np.float32)
    j = np.minimum(j0, np.float32(c2 - c0))
    t = (j * np.asarray(c1, np.float32)).astype(np.float32)
    ae = np.abs(np.asarray(in1, np.float32) - t)
    return ae, ae.sum(axis=1, keepdims=True)


def _sq_ref(in0, in1, c0, c1, c2):
    """numpy reference for SQSUM_ANT."""
    x = np.asarray(in0, np.float32)
    sq = x * x
    return sq, sq.sum(axis=1, keepdims=True)


def _register_ops():
    """Register the custom DVE ops in concourse's table (runtime append;
    the uop programs are compiled into the per-NEFF DVE table)."""
    import concourse.dve_ops as D
    from concourse.dve_spec import (
        Spec, Src0, Src1, C0, C1, C2, maxx, minn, sq, Bin, AluOp, lower,
        _has_src1,
    )
    from concourse.dve_uop import DveOpSpec

    def reg(name, spec):
        if name in D._SUB_OPCODE_FOR_NAME:
            for op in D.OPS:
                if op.name == name:
                    return op
        row = D._CUSTOM_DVE_ROW_BASE + len(D.OPS)
        assert row < 0x20, "custom DVE row overflow"
        shas = {}
        for ver in ("v3", "v4"):
            s = DveOpSpec(name=name, opcode=row,
                          uops=lower(spec, ver=ver), rd1_en=_has_src1(spec))
            shas[ver] = s.sha(ver)
        op = D.DveOp(name, spec, subdim=False, uops_sha=shas)
        D.OPS.append(op)
        D._SUB_OPCODE_FOR_NAME[name] = row
        D.CUSTOM_DVE_SPECS[name] = spec
        return op

    v = Src0 + C0              # 2^23 + round(z)   (f32 internal)
    v2 = maxx(v, C0)           # clip low: round(z) >= 0
    j0 = v2 - C2               # max(round(z),0) - 7
    j = minn(j0, C2 - C0)      # min(..., 7)  (C2-C0 = 7, auto-hoisted)
    t = j * C1                 # s * jc7
    ae = Bin(AluOp.ABSOLUTE_DIFF, Src1, t)   # |w - s*jc7|
    vq = reg("VQ_LOSS_ANT",
             Spec(body=ae, accum=AluOp.ADD, reference=_vq_ref))
    sqs = reg("SQSUM_ANT",
              Spec(body=sq(Src0), accum=AluOp.ADD, reference=_sq_ref))
    return vq, sqs


def _build():
    vq, sqs = _register_ops()
    nc = bacc.Bacc("TRN2", target_bir_lowering=False, debug=False,
                   num_devices=NCORES)
    w_ext = nc.dram_tensor("w", [CSH, K], f32, kind="ExternalInput")
    s_ext = nc.dram_tensor("s", [CSH, 1], f32, kind="ExternalInput")
    out_ext = nc.dram_tensor("out", [P, NBLK], f32, kind="ExternalOutput")

    with tile.TileContext(nc) as tc:
        with (
            tc.tile_pool(name="w32p", bufs=2) as w32p,
            tc.tile_pool(name="w16p", bufs=2) as w16p,
            tc.tile_pool(name="zp", bufs=2) as zp,
            tc.tile_pool(name="minis", bufs=2) as minis,
            tc.tile_pool(name="outp", bufs=1) as outp,
        ):
            out_sb = outp.tile([P, NBLK], f32)
            seven = outp.tile([P, 1], f32)
            nc.vector.memset(seven[:], 7.0)

            for b in range(NBLK):
                rows = slice(b * P, (b + 1) * P)
                sblk = minis.tile([P, 1], f32)
                nc.sync.dma_start(sblk[:], s_ext[rows, :])

                w16 = w16p.tile([P, K], f16)
                z = zp.tile([P, K], f16, tag="z")
                st = minis.tile([P, 6], f32)   # su0 su1 sq0 sq1 sqD
                for c in range(NCH):
                    sl = slice(c * CH, (c + 1) * CH)
                    w32 = w32p.tile([P, CH], f32, tag="w32")
                    nc.sync.dma_start(w32[:], w_ext[rows, sl])
                    # conversion pass carries Sum(w)
                    nc.scalar.activation(w16[:, sl], w32[:], Act.Copy,
                                         accum_out=st[:, c:c + 1])
                    # square pass carries Sum(w^2); scratch into z
                    hi = min(SQA, (c + 1) * CH)
                    if hi > c * CH:
                        nc.scalar.activation(
                            z[:, c * CH:hi], w32[:, 0:hi - c * CH],
                            Act.Square,
                            accum_out=st[:, 2 + c:3 + c])
                # remainder of Sum(w^2) on DVE from w_f16
                nc.vector._custom_dve(sqs, out=z[:, SQA:K],
                                      in0=w16[:, SQA:K],
                                      accum_out=st[:, 4:5])

                # per-channel params: r = 1/step, nb1 = mean*r - 7
                SU = minis.tile([P, 1], f32)
                nc.vector.tensor_reduce(SU[:], st[:, 0:2],
                                        mybir.AxisListType.X, Alu.add)
                SQ = minis.tile([P, 1], f32)
                nc.vector.tensor_reduce(SQ[:], st[:, 2:5],
                                        mybir.AxisListType.X, Alu.add)
                mean = minis.tile([P, 1], f32)
                nc.vector.tensor_scalar(mean[:], SU[:], INV_K, None, Alu.mult)
                E2 = minis.tile([P, 1], f32)
                nc.vector.tensor_scalar(E2[:], SQ[:], INV_K, None, Alu.mult)
                nvar = minis.tile([P, 1], f32)
                # nvar = mean*mean - E2  (= -var_biased)
                nc.vector.scalar_tensor_tensor(nvar[:], mean[:], mean[:],
                                               E2[:], Alu.mult, Alu.subtract)
                step = minis.tile([P, 1], f32)
                # step = sqrt(K2*var_b) = Sqrt(-K2 * nvar)
                nc.scalar.activation(step[:], nvar[:], Act.Sqrt,
                                     bias=0.0, scale=-K2)
                r = minis.tile([P, 1], f32)
                nc.vector.reciprocal(r[:], step[:])
                nb1 = minis.tile([P, 1], f32)
                # nb1 = mean*r - 7
                nc.vector.scalar_tensor_tensor(nb1[:], mean[:], r[:],
                                               seven[:], Alu.mult,
                                               Alu.subtract)

                # z = w16*r - nb1   (ts dual, f16, 4x)
                nc.vector.tensor_scalar(z[:], w16[:], r[:], nb1[:],
                                        Alu.mult, Alu.subtract)
                # fused loss: |w - s*(clip(round(z),0,14)-7)|, accum -> out
                nc.vector._custom_dve(vq, out=z[:], in0=z[:], in1=w16[:],
                                      s0=RND, s1=sblk[:], imm2=RND7,
                                      accum_out=out_sb[:, b:b + 1])

            nc.sync.dma_start(out_ext[:], out_sb[:])

    nc.compile()
    return nc


def _get_program():
    global _PROGRAM
    if _PROGRAM is None:
        _PROGRAM = _build()
    return _PROGRAM


def kernel(weight, scale):
    w = np.ascontiguousarray(np.asarray(weight, dtype=np.float32))
    s = np.ascontiguousarray(
        np.asarray(scale, dtype=np.float32)).reshape(CFULL, 1)
    assert w.shape == (CFULL, K), w.shape

    nc = _get_program()
    in_maps = [
        {"w": w[i * CSH:(i + 1) * CSH], "s": s[i * CSH:(i + 1) * CSH]}
        for i in range(NCORES)
    ]
    res = run_bass_kernel_spmd(nc, in_maps, list(range(NCORES)))
    total = 0.0
    for i in range(NCORES):
        total += res.results[i]["out"].astype(np.float64).sum()
    return np.float32(total)


# revision 7
# speedup vs baseline: 1.0539x; 1.0539x over previous
"""BalancedWeightClusterLoss on 8 Trainium2 NeuronCores (Bass/Tile).

Reference computation (per channel c of weight [C, K], scale [C]):
    mean, std(ddof=1) over K
    lower = mean - 2*std ; step = 4*std/15
    idx = clip((w - lower)/step, 0, 14) -> int (trunc == floor here)
    target = scale * (idx - 7)
    loss = sum |w - target|

Kernel derivation (per channel; r = 1/step, nb1 = mean*r - 7):
    idx = floor((w-lower)*r) = round(w*r - nb1)      (round(x-.5)==floor(x))
    jc7 = clip(round(z), 0, 14) - 7,  z = w*r - nb1
    loss = sum |w - s*jc7|

Engine split (both ~26us/block, pipelined across 4 row-blocks):
    ACT:  Copy(w_f32 -> w_f16) + riding accumulator => Sum(w)
          Square(w_f32, cols [0:SQA]) + accumulator => most of Sum(w^2)
          Sqrt for step (tiny)
    DVE:  SQSUM custom op on cols [SQA:K] of w_f16  => rest of Sum(w^2)
          z = w_f16*r - nb1     (tensor_scalar dual, f16, 4x mode)
          VQ_LOSS_ANT custom op (registered at import):
             j = min(max(z + 2^23, 2^23) - (2^23+7), 7)  [f32 internal round]
             out = |w - s*j|, accum_out = per-channel loss sum
          (1 elem/cycle; the loss reduction rides the same pass for free)
    All big reductions ride engine accumulators; no separate reduce passes.
    Square-pass scratch and custom-op garbage both land in the z tile.

Sharding: channels 4096 -> 512 per core (8 cores) x 4 row-blocks of 128
partitions. w is read from HBM exactly once (memory roofline ~94us/core).
Host sums the 8 x [128, 4] partial losses in float64.
"""
import numpy as np

import concourse.bacc as bacc
import concourse.tile as tile
from concourse import mybir
from concourse.bass_utils import run_bass_kernel_spmd

f32 = mybir.dt.float32
f16 = mybir.dt.float16
Alu = mybir.AluOpType
Act = mybir.ActivationFunctionType

# problem shape (hardcoded per contest contract)
CFULL, K = 4096, 16384
NCORES = 8
CSH = CFULL // NCORES          # 512 channels per core
P = 128                        # SBUF partitions
NBLK = CSH // P                # 4 row-blocks per core
CH = 4096                      # phase-1 chunk (f32 DMA + ACT passes)
NCH = K // CH                  # 4
SQA = 13312                    # Sum(w^2): ACT takes [0:SQA], DVE [SQA:K]

RND = float(2 ** 23)           # f32 round-to-int bias
RND7 = float(2 ** 23 + 7)
INV_K = 1.0 / K
# step^2 = K2 * var_biased ; var_b = E[w^2] - mean^2
K2 = (4.0 / 15.0) ** 2 * (K / (K - 1.0))

_PROGRAM = None


def _vq_ref(in0, in1, c0, c1, c2):
    """numpy reference for VQ_LOSS_ANT (CoreSim executes this)."""
    z32 = np.asarray(in0, np.float32)
    v = (z32 + np.float32(c0)).astype(np.float32)
    v2 = np.maximum(v, np.float32(c0))
    j0 = (v2 - np.float32(c2)).astype(np.float32)
    j = np.minimum(j0, np.float32(c2 - c0))
    t = (j * np.asarray(c1, np.float32)).astype(np.float32)
    ae = np.abs(np.asarray(in1, np.float32) - t)
    return ae, ae.sum(axis=1, keepdims=True)


def _sq_ref(in0, in1, c0, c1, c2):
    """numpy reference for SQSUM_ANT."""
    x = np.asarray(in0, np.float32)
    sq = x * x
    return sq, sq.sum(axis=1, keepdims=True)


def _register_ops():
    """Register the custom DVE ops in concourse's table (runtime append;
    the uop programs are compiled into the per-NEFF DVE table)."""
    import concourse.dve_ops as D
    from concourse.dve_spec import (
        Spec, Src0, Src1, C0, C1, C2, maxx, minn, sq, Bin, AluOp, lower,
        _has_src1,
    )
    from concourse.dve_uop import DveOpSpec

    def reg(name, spec):
        if name in D._SUB_OPCODE_FOR_NAME:
            for op in D.OPS:
                if op.name == name:
                    return op
        row = D._CUSTOM_DVE_ROW_BASE + len(D.OPS)
        assert row < 0x20, "custom DVE row overflow"
        shas = {}
        for ver in ("v3", "v4"):
            s = DveOpSpec(name=name, opcode=row,
                          uops=lower(spec, ver=ver), rd1_en=_has_src1(spec))
            shas[ver] = s.sha(ver)
        op = D.DveOp(name, spec, subdim=False, uops_sha=shas)
        D.OPS.append(op)
        D._SUB_OPCODE_FOR_NAME[name] = row
        D.CUSTOM_DVE_SPECS[name] = spec
        return op

    v = Src0 + C0              # 2^23 + round(z)   (f32 internal)
    v2 = maxx(v, C0)           # clip low: round(z) >= 0
    j0 = v2 - C2               # max(round(z),0) - 7
    j = minn(j0, C2 - C0)      # min(..., 7)  (C2-C0 = 7, auto-hoisted)
    t = j * C1                 # s * jc7
    ae = Bin(AluOp.ABSOLUTE_DIFF, Src1, t)   # |w - s*jc7|
    vq = reg("VQ_LOSS_ANT",
             Spec(body=ae, accum=AluOp.ADD, reference=_vq_ref))
    sqs = reg("SQSUM_ANT",
              Spec(body=sq(Src0), accum=AluOp.ADD, reference=_sq_ref))
    return vq, sqs


def _build():
    vq, sqs = _register_ops()
    nc = bacc.Bacc("TRN2", target_bir_lowering=False, debug=False,
                   num_devices=NCORES)
    w_ext = nc.dram_tensor("w", [CSH, K], f32, kind="ExternalInput")
    s_ext = nc.dram_tensor("s", [CSH, 1], f32, kind="ExternalInput")
    out_ext = nc.dram_tensor("out", [P, NBLK], f32, kind="ExternalOutput")

    with tile.TileContext(nc) as tc:
        with (
            tc.tile_pool(name="w32p", bufs=4) as w32p,
            tc.tile_pool(name="w16p", bufs=2) as w16p,
            tc.tile_pool(name="zp", bufs=2) as zp,
            tc.tile_pool(name="minis", bufs=2) as minis,
            tc.tile_pool(name="outp", bufs=1) as outp,
        ):
            out_sb = outp.tile([P, NBLK], f32)
            seven = outp.tile([P, 1], f32)
            nc.vector.memset(seven[:], 7.0)

            for b in range(NBLK):
                rows = slice(b * P, (b + 1) * P)
                sblk = minis.tile([P, 1], f32)
                nc.sync.dma_start(sblk[:], s_ext[rows, :])

                w16 = w16p.tile([P, K], f16)
                z = zp.tile([P, K], f16, tag="z")
                st = minis.tile([P, 9], f32)   # su[0:4] sq[4:8] sqD[8]
                for c in range(NCH):
                    sl = slice(c * CH, (c + 1) * CH)
                    w32 = w32p.tile([P, CH], f32, tag="w32")
                    nc.sync.dma_start(w32[:], w_ext[rows, sl])
                    # conversion pass carries Sum(w)
                    nc.scalar.activation(w16[:, sl], w32[:], Act.Copy,
                                         accum_out=st[:, c:c + 1])
                    # square pass carries Sum(w^2); scratch into z
                    hi = min(SQA, (c + 1) * CH)
                    if hi > c * CH:
                        nc.scalar.activation(
                            z[:, c * CH:hi], w32[:, 0:hi - c * CH],
                            Act.Square,
                            accum_out=st[:, 4 + c:5 + c])
                # remainder of Sum(w^2) on DVE from w_f16
                nc.vector._custom_dve(sqs, out=z[:, SQA:K],
                                      in0=w16[:, SQA:K],
                                      accum_out=st[:, 8:9])

                # per-channel params: r = 1/step, nb1 = mean*r - 7
                SU = minis.tile([P, 1], f32)
                nc.vector.tensor_reduce(SU[:], st[:, 0:4],
                                        mybir.AxisListType.X, Alu.add)
                SQ = minis.tile([P, 1], f32)
                nc.vector.tensor_reduce(SQ[:], st[:, 4:9],
                                        mybir.AxisListType.X, Alu.add)
                mean = minis.tile([P, 1], f32)
                nc.vector.tensor_scalar(mean[:], SU[:], INV_K, None, Alu.mult)
                E2 = minis.tile([P, 1], f32)
                nc.vector.tensor_scalar(E2[:], SQ[:], INV_K, None, Alu.mult)
                nvar = minis.tile([P, 1], f32)
                # nvar = mean*mean - E2  (= -var_biased)
                nc.vector.scalar_tensor_tensor(nvar[:], mean[:], mean[:],
                                               E2[:], Alu.mult, Alu.subtract)
                step = minis.tile([P, 1], f32)
                # step = sqrt(K2*var_b) = Sqrt(-K2 * nvar)
                nc.scalar.activation(step[:], nvar[:], Act.Sqrt,
                                     bias=0.0, scale=-K2)
                r = minis.tile([P, 1], f32)
                nc.vector.reciprocal(r[:], step[:])
                nb1 = minis.tile([P, 1], f32)
                # nb1 = mean*r - 7
                nc.vector.scalar_tensor_tensor(nb1[:], mean[:], r[:],
                                               seven[:], Alu.mult,
                                               Alu.subtract)

                # z = w16*r - nb1   (ts dual, f16, 4x)
                nc.vector.tensor_scalar(z[:], w16[:], r[:], nb1[:],
                                        Alu.mult, Alu.subtract)
                # fused loss: |w - s*(clip(round(z),0,14)-7)|, accum -> out
                nc.vector._custom_dve(vq, out=z[:], in0=z[:], in1=w16[:],
                                      s0=RND, s1=sblk[:], imm2=RND7,
                                      accum_out=out_sb[:, b:b + 1])

            nc.sync.dma_start(out_ext[:], out_sb[:])

    nc.compile()
    return nc


def _get_program():
    global _PROGRAM
    if _PROGRAM is None:
        _PROGRAM = _build()
    return _PROGRAM


def kernel(weight, scale):
    w = np.ascontiguousarray(np.asarray(weight, dtype=np.float32))
    s = np.ascontiguousarray(
        np.asarray(scale, dtype=np.float32)).reshape(CFULL, 1)
    assert w.shape == (CFULL, K), w.shape

    nc = _get_program()
    in_maps = [
        {"w": w[i * CSH:(i + 1) * CSH], "s": s[i * CSH:(i + 1) * CSH]}
        for i in range(NCORES)
    ]
    res = run_bass_kernel_spmd(nc, in_maps, list(range(NCORES)))
    total = 0.0
    for i in range(NCORES):
        total += res.results[i]["out"].astype(np.float64).sum()
    return np.float32(total)
